# revision 2
# baseline (speedup 1.0000x reference)
"""BitNet int8 x int2-packed GEMM on 8 Trainium2 NeuronCores.

Reference computation:
    W = unpack_i2u(B)            # [N, K] int8, values in {0,1,2,3}
    C = A @ W.T  (int32 accum)   # [M, N]

with M, N, K = 1024, 11008, 4096;  A int8 [M, K];  B packed int8 [N, K//4].
Packing interleave: within each group of 4 bytes (16 weights),
    W[n, 16g + 4i + j] = (byte(B[n, 4g+j]) >> 2i) & 3.

Strategy (tensor-parallel, per sharding hint):
  * Shard B along N across the 8 cores (1376 columns of C each), replicate A.
  * Host-side layout prep only (no dequant on host): B is transposed to
    [K/4, N] so each core's shard loads with unit-stride DMA, and A is
    transposed + column-permuted so that the on-device unpack order of the
    2-bit weights matches A's contraction order (the GEMM is invariant
    under a shared permutation of K).
  * On device: packed bytes are expanded with fused DVE ops
    (x >> 2i) & 3 straight to bf16; A is cast int8->bf16 once on the
    scalar engine.  The GEMM runs in bf16 with fp32 PSUM accumulation,
    which is exact here: |products| <= 384 and |partial sums| < 2^21 << 2^24.
  * PSUM results are cast fp32->int32 and DMA'd out; the host concatenates
    the 8 column shards.

K-permutation: define k' = i*(K/4) + kc  (kc = packed byte index, i = shift).
Unpacking byte-tile rows kc with shift i yields weight rows k' directly, and
A is pre-permuted on host with sigma(k') = 16*(kc//4) + 4i + (kc%4) so both
operands use the same contraction order.
"""

import numpy as np

M, K, N = 1024, 4096, 11008
NCORES = 8
NSHARD = N // NCORES  # 1376

_prog_cache: dict = {}


def _build(m, k, nshard, ncores):
    from contextlib import ExitStack

    import concourse.tile as tile
    from concourse import bacc, mybir

    kt_n = k // 128  # number of 128-row k'-tiles (32)
    pk_n = k // 512  # number of 128-row packed-byte tiles (8)
    mt_n = m // 128  # number of output row tiles (8)

    n_tiles = []
    n0 = 0
    while n0 < nshard:
        nw = min(512, nshard - n0)
        n_tiles.append((n0, nw))
        n0 += nw

    nc = bacc.Bacc("TRN2", target_bir_lowering=False, debug=False, num_devices=ncores)
    a_t = nc.dram_tensor("a_t", [k, m], mybir.dt.int8, kind="ExternalInput").ap()
    b_t = nc.dram_tensor("b_t", [k // 4, nshard], mybir.dt.int8, kind="ExternalInput").ap()
    c = nc.dram_tensor("c", [m, nshard], mybir.dt.int32, kind="ExternalOutput").ap()

    with tile.TileContext(nc) as tc, ExitStack() as ctx:
        apool = ctx.enter_context(tc.tile_pool(name="a_res", bufs=1))
        a8pool = ctx.enter_context(tc.tile_pool(name="a_stage", bufs=2))
        wpool = ctx.enter_context(tc.tile_pool(name="w", bufs=2))
        ppool = ctx.enter_context(tc.tile_pool(name="packed", bufs=4))
        opool = ctx.enter_context(tc.tile_pool(name="out", bufs=4))
        pspool = ctx.enter_context(tc.tile_pool(name="ps", bufs=8, space="PSUM"))

        # Resident bf16 copy of A^T (k' on partitions): k'-tile t lives at
        # columns [t*m, (t+1)*m).
        a_all = apool.tile([128, kt_n * m], mybir.dt.bfloat16)
        for t in range(kt_n):
            a8 = a8pool.tile([128, m], mybir.dt.int8, tag="a8")
            nc.sync.dma_start(a8[:], a_t[t * 128 : (t + 1) * 128, :])
            nc.scalar.copy(a_all[:, t * m : (t + 1) * m], a8[:])

        for n0, nw in n_tiles:
            # Unpacked weights for this n-slice: k'-tile kt at columns
            # [kt*512, kt*512+nw).  The fused shift+and must stay int8->int8
            # (walrus: bitvec ops can't cast), so a separate GpSimd copy does
            # the int8->bf16 conversion.
            w_all = wpool.tile([128, kt_n * 512], mybir.dt.bfloat16, tag="w")
            for t in range(pk_n):
                p8 = ppool.tile([128, 512], mybir.dt.int8, tag="p8")
                nc.sync.dma_start(p8[:, :nw], b_t[t * 128 : (t + 1) * 128, n0 : n0 + nw])
                for i in range(4):
                    kt = i * pk_n + t
                    w8 = ppool.tile([128, 512], mybir.dt.int8, tag="w8")
                    nc.vector.tensor_scalar(
                        w8[:, :nw],
                        p8[:, :nw],
                        2 * i,
                        3,
                        op0=mybir.AluOpType.logical_shift_right,
                        op1=mybir.AluOpType.bitwise_and,
                    )
                    nc.gpsimd.tensor_copy(w_all[:, kt * 512 : kt * 512 + nw], w8[:, :nw])
            for mt in range(mt_n):
                ps = pspool.tile([128, 512], mybir.dt.float32, tag="ps")
                for kt in range(kt_n):
                    nc.tensor.matmul(
                        ps[:, :nw],
                        a_all[:, kt * m + mt * 128 : kt * m + mt * 128 + 128],
                        w_all[:, kt * 512 : kt * 512 + nw],
                        start=(kt == 0),
                        stop=(kt == kt_n - 1),
                    )
                o = opool.tile([128, 512], mybir.dt.int32, tag="o")
                nc.vector.tensor_copy(o[:, :nw], ps[:, :nw])
                nc.sync.dma_start(c[mt * 128 : (mt + 1) * 128, n0 : n0 + nw], o[:, :nw])

    nc.compile()
    return nc


def _get_program():
    key = (M, K, NSHARD, NCORES)
    if key not in _prog_cache:
        _prog_cache[key] = _build(*key)
    return _prog_cache[key]


def _prep_inputs(A, B):
    A = np.ascontiguousarray(np.asarray(A, dtype=np.int8))
    B = np.ascontiguousarray(np.asarray(B, dtype=np.int8))
    # A^T with k-permutation sigma(k' = i*(K/4) + 4g + j) = 16g + 4i + j.
    a_perm_t = np.ascontiguousarray(
        A.reshape(M, K // 16, 4, 4).transpose(2, 1, 3, 0).reshape(K, M)
    )
    b_t = np.ascontiguousarray(B.T)  # [K//4, N]
    return a_perm_t, b_t


def kernel(A, B):
    from concourse.bass_utils import run_bass_kernel_spmd

    a_perm_t, b_t = _prep_inputs(A, B)
    nc = _get_program()
    in_maps = [
        {
            "a_t": a_perm_t,
            "b_t": np.ascontiguousarray(b_t[:, ci * NSHARD : (ci + 1) * NSHARD]),
        }
        for ci in range(NCORES)
    ]
    res = run_bass_kernel_spmd(nc, in_maps, core_ids=list(range(NCORES)))
    return np.concatenate([res.results[ci]["c"] for ci in range(NCORES)], axis=1)


# revision 6
# speedup vs baseline: 1.4548x; 1.4548x over previous
"""BitNet int8 x int2-packed GEMM on 8 Trainium2 NeuronCores.

Reference computation:
    W = unpack_i2u(B)            # [N, K] int8, values in {0,1,2,3}
    C = A @ W.T  (int32 accum)   # [M, N]

with M, N, K = 1024, 11008, 4096;  A int8 [M, K];  B packed int8 [N, K//4].
Packing interleave: within each group of 4 bytes (16 weights),
    W[n, 16g + 4i + j] = (byte(B[n, 4g+j]) >> 2i) & 3.

Strategy (tensor-parallel, per sharding hint):
  * Shard B along N across the 8 cores (1376 columns of C each), replicate A.
  * Host-side layout prep only (no dequant on host): B is transposed to
    [K/4, N] so each core's shard loads with unit-stride DMA, and A is
    transposed + column-permuted so that the on-device unpack order of the
    2-bit weights matches A's contraction order (the GEMM is invariant
    under a shared permutation of K).
  * On device: packed bytes are expanded with fused DVE ops
    (x >> 2i) & 3 straight to bf16; A is cast int8->bf16 once on the
    scalar engine.  The GEMM runs in bf16 with fp32 PSUM accumulation,
    which is exact here: |products| <= 384 and |partial sums| < 2^21 << 2^24.
  * PSUM results are cast fp32->int32 and DMA'd out; the host concatenates
    the 8 column shards.

K-permutation: define k' = i*(K/4) + kc  (kc = packed byte index, i = shift).
Unpacking byte-tile rows kc with shift i yields weight rows k' directly, and
A is pre-permuted on host with sigma(k') = 16*(kc//4) + 4i + (kc%4) so both
operands use the same contraction order.
"""

import numpy as np

M, K, N = 1024, 4096, 11008
NCORES = 8
NSHARD = N // NCORES  # 1376

_prog_cache: dict = {}


def _build(m, k, nshard, ncores):
    from contextlib import ExitStack

    import concourse.tile as tile
    from concourse import bacc, mybir

    kt_n = k // 128  # number of 128-row k'-tiles (32)
    pk_n = k // 512  # number of 128-row packed-byte tiles (8)
    mt_n = m // 128  # number of output row tiles (8)

    n_tiles = []
    n0 = 0
    while n0 < nshard:
        nw = min(512, nshard - n0)
        n_tiles.append((n0, nw))
        n0 += nw

    nc = bacc.Bacc("TRN2", target_bir_lowering=False, debug=False, num_devices=ncores)
    a_t = nc.dram_tensor("a_t", [k, m], mybir.dt.int8, kind="ExternalInput").ap()
    # Packed bytes as int32 words (4 n-columns per word) so the unpack runs
    # 4 bytes per DVE lane-element.
    b_t = nc.dram_tensor(
        "b_t", [k // 4, nshard // 4], mybir.dt.int32, kind="ExternalInput"
    ).ap()
    c = nc.dram_tensor("c", [m, nshard], mybir.dt.int32, kind="ExternalOutput").ap()

    with tile.TileContext(nc) as tc, ExitStack() as ctx:
        apool = ctx.enter_context(tc.tile_pool(name="a_res", bufs=1))
        a8pool = ctx.enter_context(tc.tile_pool(name="a_stage", bufs=4))
        wpool = ctx.enter_context(tc.tile_pool(name="w", bufs=2))
        ppool = ctx.enter_context(tc.tile_pool(name="packed", bufs=4))
        opool = ctx.enter_context(tc.tile_pool(name="out", bufs=8))
        pspool = ctx.enter_context(tc.tile_pool(name="ps", bufs=8, space="PSUM"))

        # Resident bf16 copy of A^T (k' on partitions): k'-tile t lives at
        # columns [t*m, (t+1)*m).  Casts alternate ACT/DVE.
        a_all = apool.tile([128, kt_n * m], mybir.dt.bfloat16)
        for t in range(kt_n):
            a8 = a8pool.tile([128, m], mybir.dt.int8, tag="a8")
            nc.sync.dma_start(a8[:], a_t[t * 128 : (t + 1) * 128, :])
            if t % 2 == 0:
                nc.scalar.copy(a_all[:, t * m : (t + 1) * m], a8[:])
            else:
                nc.vector.tensor_copy(a_all[:, t * m : (t + 1) * m], a8[:])

        for n0, nw in n_tiles:
            # Unpacked weights for this n-slice: k'-tile kt at columns
            # [kt*512, kt*512+nw).  The fused shift+and must keep its dtype
            # (walrus: bitvec ops can't cast), and runs on int32 words with a
            # per-byte mask: (word >> 2i) & 0x03030303 extracts weight i of
            # each of the 4 packed bytes (shifted bits never cross into a
            # masked lane).  A separate copy then casts the int8 view of the
            # result to bf16, alternating ACT/DVE.
            w_all = wpool.tile([128, kt_n * 512], mybir.dt.bfloat16, tag="w")
            for t in range(pk_n):
                p32 = ppool.tile([128, 128], mybir.dt.int32, tag="p32")
                nc.sync.dma_start(
                    p32[:, : nw // 4],
                    b_t[t * 128 : (t + 1) * 128, n0 // 4 : (n0 + nw) // 4],
                )
                for i in range(4):
                    kt = i * pk_n + t
                    w32 = ppool.tile([128, 128], mybir.dt.int32, tag="w32")
                    nc.vector.tensor_scalar(
                        w32[:, : nw // 4],
                        p32[:, : nw // 4],
                        2 * i,
                        0x03030303,
                        op0=mybir.AluOpType.logical_shift_right,
                        op1=mybir.AluOpType.bitwise_and,
                    )
                    src = w32[:, : nw // 4].bitcast(mybir.dt.int8)
                    dst = w_all[:, kt * 512 : kt * 512 + nw]
                    if i % 2 == 0:
                        nc.scalar.copy(dst, src)
                    else:
                        nc.vector.tensor_copy(dst, src)
            # kt-outer / mt-inner: all 8 PSUM banks accumulate in parallel, so
            # the PE can start as soon as the first A/W k'-tiles are ready and
            # never stalls on a single cast chain.
            ps_tiles = [
                pspool.tile([128, 512], mybir.dt.float32, tag="ps", name="ps")
                for _ in range(mt_n)
            ]
            for kt in range(kt_n):
                for mt in range(mt_n):
                    nc.tensor.matmul(
                        ps_tiles[mt][:, :nw],
                        a_all[:, kt * m + mt * 128 : kt * m + mt * 128 + 128],
                        w_all[:, kt * 512 : kt * 512 + nw],
                        start=(kt == 0),
                        stop=(kt == kt_n - 1),
                    )
            for mt in range(mt_n):
                o = opool.tile([128, 512], mybir.dt.int32, tag="o")
                if mt % 2 == 0:
                    nc.vector.tensor_copy(o[:, :nw], ps_tiles[mt][:, :nw])
                else:
                    nc.scalar.copy(o[:, :nw], ps_tiles[mt][:, :nw])
                nc.sync.dma_start(c[mt * 128 : (mt + 1) * 128, n0 : n0 + nw], o[:, :nw])

    nc.compile()
    return nc


def _get_program():
    key = (M, K, NSHARD, NCORES)
    if key not in _prog_cache:
        _prog_cache[key] = _build(*key)
    return _prog_cache[key]


def _prep_inputs(A, B):
    A = np.ascontiguousarray(np.asarray(A, dtype=np.int8))
    B = np.ascontiguousarray(np.asarray(B, dtype=np.int8))
    # A^T with k-permutation sigma(k' = i*(K/4) + 4g + j) = 16g + 4i + j.
    a_perm_t = np.ascontiguousarray(
        A.reshape(M, K // 16, 4, 4).transpose(2, 1, 3, 0).reshape(K, M)
    )
    b_t = np.ascontiguousarray(B.T)  # [K//4, N] int8
    return a_perm_t, b_t


def kernel(A, B):
    from concourse.bass_utils import run_bass_kernel_spmd

    a_perm_t, b_t = _prep_inputs(A, B)
    nc = _get_program()
    in_maps = [
        {
            "a_t": a_perm_t,
            "b_t": np.ascontiguousarray(
                b_t[:, ci * NSHARD : (ci + 1) * NSHARD]
            ).view(np.int32),
        }
        for ci in range(NCORES)
    ]
    res = run_bass_kernel_spmd(nc, in_maps, core_ids=list(range(NCORES)))
    return np.concatenate([res.results[ci]["c"] for ci in range(NCORES)], axis=1)


# revision 8
# speedup vs baseline: 1.6071x; 1.1047x over previous
"""BitNet int8 x int2-packed GEMM on 8 Trainium2 NeuronCores.

Reference computation:
    W = unpack_i2u(B)            # [N, K] int8, values in {0,1,2,3}
    C = A @ W.T  (int32 accum)   # [M, N]

with M, N, K = 1024, 11008, 4096;  A int8 [M, K];  B packed int8 [N, K//4].
Packing interleave: within each group of 4 bytes (16 weights),
    W[n, 16g + 4i + j] = (byte(B[n, 4g+j]) >> 2i) & 3.

Strategy (tensor-parallel, per sharding hint):
  * Shard B along N across the 8 cores (1376 columns of C each), replicate A.
  * Host-side layout prep only (no dequant on host): B is transposed to
    [K/4, N] so each core's shard loads with unit-stride DMA, and A is
    transposed + column-permuted so that the on-device unpack order of the
    2-bit weights matches A's contraction order (the GEMM is invariant
    under a shared permutation of K).
  * On device: packed bytes are expanded with fused DVE ops
    (x >> 2i) & 3 straight to bf16; A is cast int8->bf16 once on the
    scalar engine.  The GEMM runs in bf16 with fp32 PSUM accumulation,
    which is exact here: |products| <= 384 and |partial sums| < 2^21 << 2^24.
  * PSUM results are cast fp32->int32 and DMA'd out; the host concatenates
    the 8 column shards.

K-permutation: define k' = i*(K/4) + kc  (kc = packed byte index, i = shift).
Unpacking byte-tile rows kc with shift i yields weight rows k' directly, and
A is pre-permuted on host with sigma(k') = 16*(kc//4) + 4i + (kc%4) so both
operands use the same contraction order.
"""

import numpy as np

M, K, N = 1024, 4096, 11008
NCORES = 8
NSHARD = N // NCORES  # 1376

_prog_cache: dict = {}


def _build(m, k, nshard, ncores):
    from contextlib import ExitStack

    import concourse.tile as tile
    from concourse import bacc, mybir

    kt_n = k // 128  # number of 128-row k'-tiles (32)
    pk_n = k // 512  # number of 128-row packed-byte tiles (8)
    mt_n = m // 128  # number of output row tiles (8)

    n_tiles = []
    n0 = 0
    while n0 < nshard:
        nw = min(512, nshard - n0)
        n_tiles.append((n0, nw))
        n0 += nw

    nc = bacc.Bacc("TRN2", target_bir_lowering=False, debug=False, num_devices=ncores)
    a_t = nc.dram_tensor("a_t", [k, m], mybir.dt.int8, kind="ExternalInput").ap()
    # Packed bytes as int32 words (4 n-columns per word) so the unpack runs
    # 4 bytes per DVE lane-element.
    b_t = nc.dram_tensor(
        "b_t", [k // 4, nshard // 4], mybir.dt.int32, kind="ExternalInput"
    ).ap()
    c = nc.dram_tensor("c", [m, nshard], mybir.dt.int32, kind="ExternalOutput").ap()

    with tile.TileContext(nc) as tc, ExitStack() as ctx:
        apool = ctx.enter_context(tc.tile_pool(name="a_res", bufs=1))
        a8pool = ctx.enter_context(tc.tile_pool(name="a_stage", bufs=4))
        wpool = ctx.enter_context(tc.tile_pool(name="w", bufs=2))
        ppool = ctx.enter_context(tc.tile_pool(name="packed", bufs=4))
        opool = ctx.enter_context(tc.tile_pool(name="out", bufs=8))
        pspool = ctx.enter_context(tc.tile_pool(name="ps", bufs=8, space="PSUM"))

        # Resident bf16 copy of A^T (k' on partitions): k'-tile t lives at
        # columns [t*m, (t+1)*m).  Casts alternate ACT/DVE.
        a_all = apool.tile([128, kt_n * m], mybir.dt.bfloat16)
        for t in range(kt_n):
            a8 = a8pool.tile([128, m], mybir.dt.int8, tag="a8")
            nc.sync.dma_start(a8[:], a_t[t * 128 : (t + 1) * 128, :])
            if t % 2 == 0:
                nc.scalar.copy(a_all[:, t * m : (t + 1) * m], a8[:])
            else:
                nc.vector.tensor_copy(a_all[:, t * m : (t + 1) * m], a8[:])

        for nt, (n0, nw) in enumerate(n_tiles):
            # Unpacked weights for this n-slice: k'-tile kt at columns
            # [kt*512, kt*512+nw).  The fused shift+and must keep its dtype
            # (walrus: bitvec ops can't cast), and runs on int32 words with a
            # per-byte mask: (word >> 2i) & 0x03030303 extracts weight i of
            # each of the 4 packed bytes (shifted bits never cross into a
            # masked lane).  A separate copy then casts the int8 view of the
            # result to bf16, alternating ACT/DVE per k'-tile.
            # Loop i-outer/t-inner so W tiles are produced in kt order (the
            # order the matmuls consume them).
            w_all = wpool.tile([128, kt_n * 512], mybir.dt.bfloat16, tag="w")
            p32s = []
            for t in range(pk_n):
                p32 = ppool.tile(
                    [128, 128], mybir.dt.int32, tag="p32", name="p32", bufs=16
                )
                nc.sync.dma_start(
                    p32[:, : nw // 4],
                    b_t[t * 128 : (t + 1) * 128, n0 // 4 : (n0 + nw) // 4],
                )
                p32s.append(p32)
            for i in range(4):
                for t in range(pk_n):
                    kt = i * pk_n + t
                    w32 = ppool.tile([128, 128], mybir.dt.int32, tag="w32")
                    nc.vector.tensor_scalar(
                        w32[:, : nw // 4],
                        p32s[t][:, : nw // 4],
                        2 * i,
                        0x03030303,
                        op0=mybir.AluOpType.logical_shift_right,
                        op1=mybir.AluOpType.bitwise_and,
                    )
                    src = w32[:, : nw // 4].bitcast(mybir.dt.int8)
                    dst = w_all[:, kt * 512 : kt * 512 + nw]
                    if kt % 2 == 0:
                        nc.scalar.copy(dst, src)
                    else:
                        nc.vector.tensor_copy(dst, src)
            if nt == 0:
                # kt-outer / mt-inner: all 8 PSUM banks accumulate in
                # parallel, so the PE starts as soon as the first A/W k'-tiles
                # are cast and stays busy while the A-cast ramp completes.
                ps_tiles = [
                    pspool.tile([128, 512], mybir.dt.float32, tag="ps", name="ps")
                    for _ in range(mt_n)
                ]
                for kt in range(kt_n):
                    for mt in range(mt_n):
                        nc.tensor.matmul(
                            ps_tiles[mt][:, :nw],
                            a_all[:, kt * m + mt * 128 : kt * m + mt * 128 + 128],
                            w_all[:, kt * 512 : kt * 512 + nw],
                            start=(kt == 0),
                            stop=(kt == kt_n - 1),
                        )
                for mt in range(mt_n):
                    o = opool.tile([128, 512], mybir.dt.int32, tag="o")
                    if mt % 2 == 0:
                        nc.vector.tensor_copy(o[:, :nw], ps_tiles[mt][:, :nw])
                    else:
                        nc.scalar.copy(o[:, :nw], ps_tiles[mt][:, :nw])
                    nc.sync.dma_start(
                        c[mt * 128 : (mt + 1) * 128, n0 : n0 + nw], o[:, :nw]
                    )
            else:
                # Steady state (A resident, W prefetched): mt-outer so each
                # m-tile's PSUM copy + store streams out while the next
                # m-tile's matmuls run, instead of bursting at the tile end.
                for mt in range(mt_n):
                    ps = pspool.tile([128, 512], mybir.dt.float32, tag="ps", name="ps")
                    for kt in range(kt_n):
                        nc.tensor.matmul(
                            ps[:, :nw],
                            a_all[:, kt * m + mt * 128 : kt * m + mt * 128 + 128],
                            w_all[:, kt * 512 : kt * 512 + nw],
                            start=(kt == 0),
                            stop=(kt == kt_n - 1),
                        )
                    o = opool.tile([128, 512], mybir.dt.int32, tag="o")
                    if mt % 2 == 0:
                        nc.vector.tensor_copy(o[:, :nw], ps[:, :nw])
                    else:
                        nc.scalar.copy(o[:, :nw], ps[:, :nw])
                    nc.sync.dma_start(
                        c[mt * 128 : (mt + 1) * 128, n0 : n0 + nw], o[:, :nw]
                    )

    nc.compile()
    return nc


def _get_program():
    key = (M, K, NSHARD, NCORES)
    if key not in _prog_cache:
        _prog_cache[key] = _build(*key)
    return _prog_cache[key]


def _prep_inputs(A, B):
    A = np.ascontiguousarray(np.asarray(A, dtype=np.int8))
    B = np.ascontiguousarray(np.asarray(B, dtype=np.int8))
    # A^T with k-permutation sigma(k' = i*(K/4) + 4g + j) = 16g + 4i + j.
    a_perm_t = np.ascontiguousarray(
        A.reshape(M, K // 16, 4, 4).transpose(2, 1, 3, 0).reshape(K, M)
    )
    b_t = np.ascontiguousarray(B.T)  # [K//4, N] int8
    return a_perm_t, b_t


def kernel(A, B):
    from concourse.bass_utils import run_bass_kernel_spmd

    a_perm_t, b_t = _prep_inputs(A, B)
    nc = _get_program()
    in_maps = [
        {
            "a_t": a_perm_t,
            "b_t": np.ascontiguousarray(
                b_t[:, ci * NSHARD : (ci + 1) * NSHARD]
            ).view(np.int32),
        }
        for ci in range(NCORES)
    ]
    res = run_bass_kernel_spmd(nc, in_maps, core_ids=list(range(NCORES)))
    return np.concatenate([res.results[ci]["c"] for ci in range(NCORES)], axis=1)


# revision 10
# speedup vs baseline: 1.6248x; 1.0110x over previous
"""BitNet int8 x int2-packed GEMM on 8 Trainium2 NeuronCores.

Reference computation:
    W = unpack_i2u(B)            # [N, K] int8, values in {0,1,2,3}
    C = A @ W.T  (int32 accum)   # [M, N]

with M, N, K = 1024, 11008, 4096;  A int8 [M, K];  B packed int8 [N, K//4].
Packing interleave: within each group of 4 bytes (16 weights),
    W[n, 16g + 4i + j] = (byte(B[n, 4g+j]) >> 2i) & 3.

Strategy (tensor-parallel, per sharding hint):
  * Shard B along N across the 8 cores (1376 columns of C each), replicate A.
  * Host-side layout prep only (no dequant on host): B is transposed to
    [K/4, N] so each core's shard loads with unit-stride DMA, and A is
    transposed + column-permuted so that the on-device unpack order of the
    2-bit weights matches A's contraction order (the GEMM is invariant
    under a shared permutation of K).
  * On device: packed bytes are expanded with fused DVE ops
    (x >> 2i) & 3 straight to bf16; A is cast int8->bf16 once on the
    scalar engine.  The GEMM runs in bf16 with fp32 PSUM accumulation,
    which is exact here: |products| <= 384 and |partial sums| < 2^21 << 2^24.
  * PSUM results are cast fp32->int32 and DMA'd out; the host concatenates
    the 8 column shards.

K-permutation: define k' = i*(K/4) + kc  (kc = packed byte index, i = shift).
Unpacking byte-tile rows kc with shift i yields weight rows k' directly, and
A is pre-permuted on host with sigma(k') = 16*(kc//4) + 4i + (kc%4) so both
operands use the same contraction order.
"""

import numpy as np

M, K, N = 1024, 4096, 11008
NCORES = 8
NSHARD = N // NCORES  # 1376

_prog_cache: dict = {}


def _build(m, k, nshard, ncores):
    from contextlib import ExitStack

    import concourse.tile as tile
    from concourse import bacc, mybir

    kt_n = k // 128  # number of 128-row k'-tiles (32)
    pk_n = k // 512  # number of 128-row packed-byte tiles (8)
    mt_n = m // 128  # number of output row tiles (8)

    n_tiles = []
    n0 = 0
    while n0 < nshard:
        nw = min(512, nshard - n0)
        n_tiles.append((n0, nw))
        n0 += nw

    nc = bacc.Bacc("TRN2", target_bir_lowering=False, debug=False, num_devices=ncores)
    a_t = nc.dram_tensor("a_t", [k, m], mybir.dt.int8, kind="ExternalInput").ap()
    # Packed bytes as int32 words (4 n-columns per word) so the unpack runs
    # 4 bytes per DVE lane-element.
    b_t = nc.dram_tensor(
        "b_t", [k // 4, nshard // 4], mybir.dt.int32, kind="ExternalInput"
    ).ap()
    c = nc.dram_tensor("c", [m, nshard], mybir.dt.int32, kind="ExternalOutput").ap()

    with tile.TileContext(nc) as tc, ExitStack() as ctx:
        apool = ctx.enter_context(tc.tile_pool(name="a_res", bufs=1))
        a8pool = ctx.enter_context(tc.tile_pool(name="a_stage", bufs=4))
        wpool = ctx.enter_context(tc.tile_pool(name="w", bufs=2))
        ppool = ctx.enter_context(tc.tile_pool(name="packed", bufs=4))
        opool = ctx.enter_context(tc.tile_pool(name="out", bufs=8))
        pspool = ctx.enter_context(tc.tile_pool(name="ps", bufs=8, space="PSUM"))

        # Kick off the first n-tile's packed-B loads before the A loads so the
        # weight pipeline (DMA -> shift/mask -> cast) has tile 0 ready as
        # early as possible; the first matmul needs W k'-tile 0 AND A k'-tile
        # 0, and the SP engine issues DMAs ~0.6us apart.
        first_n0, first_nw = n_tiles[0]
        first_p32s = []
        for t in range(pk_n):
            p32 = ppool.tile(
                [128, 128], mybir.dt.int32, tag="p32", name="p32", bufs=16
            )
            nc.sync.dma_start(
                p32[:, : first_nw // 4],
                b_t[t * 128 : (t + 1) * 128, first_n0 // 4 : (first_n0 + first_nw) // 4],
            )
            first_p32s.append(p32)
            if t == 0:
                # A k'-tile 0 right after B tile 0 — the two inputs of the
                # first matmul.
                a_all = apool.tile([128, kt_n * m], mybir.dt.bfloat16)
                a8 = a8pool.tile([128, m], mybir.dt.int8, tag="a8", name="a8")
                nc.sync.dma_start(a8[:], a_t[0:128, :])
                nc.scalar.copy(a_all[:, 0:m], a8[:])

        # Resident bf16 copy of A^T (k' on partitions): k'-tile t lives at
        # columns [t*m, (t+1)*m).  Casts alternate ACT/DVE.
        for t in range(1, kt_n):
            a8 = a8pool.tile([128, m], mybir.dt.int8, tag="a8", name="a8")
            nc.sync.dma_start(a8[:], a_t[t * 128 : (t + 1) * 128, :])
            if t % 2 == 0:
                nc.scalar.copy(a_all[:, t * m : (t + 1) * m], a8[:])
            else:
                nc.vector.tensor_copy(a_all[:, t * m : (t + 1) * m], a8[:])

        for nt, (n0, nw) in enumerate(n_tiles):
            # Unpacked weights for this n-slice: k'-tile kt at columns
            # [kt*512, kt*512+nw).  The fused shift+and must keep its dtype
            # (walrus: bitvec ops can't cast), and runs on int32 words with a
            # per-byte mask: (word >> 2i) & 0x03030303 extracts weight i of
            # each of the 4 packed bytes (shifted bits never cross into a
            # masked lane).  A separate copy then casts the int8 view of the
            # result to bf16, alternating ACT/DVE per k'-tile.
            # Loop i-outer/t-inner so W tiles are produced in kt order (the
            # order the matmuls consume them).
            w_all = wpool.tile([128, kt_n * 512], mybir.dt.bfloat16, tag="w")
            if nt == 0:
                p32s = first_p32s
            else:
                p32s = []
                for t in range(pk_n):
                    p32 = ppool.tile(
                        [128, 128], mybir.dt.int32, tag="p32", name="p32", bufs=16
                    )
                    nc.sync.dma_start(
                        p32[:, : nw // 4],
                        b_t[t * 128 : (t + 1) * 128, n0 // 4 : (n0 + nw) // 4],
                    )
                    p32s.append(p32)
            for i in range(4):
                for t in range(pk_n):
                    kt = i * pk_n + t
                    w32 = ppool.tile([128, 128], mybir.dt.int32, tag="w32")
                    nc.vector.tensor_scalar(
                        w32[:, : nw // 4],
                        p32s[t][:, : nw // 4],
                        2 * i,
                        0x03030303,
                        op0=mybir.AluOpType.logical_shift_right,
                        op1=mybir.AluOpType.bitwise_and,
                    )
                    src = w32[:, : nw // 4].bitcast(mybir.dt.int8)
                    dst = w_all[:, kt * 512 : kt * 512 + nw]
                    if kt % 2 == 0:
                        nc.scalar.copy(dst, src)
                    else:
                        nc.vector.tensor_copy(dst, src)
            if nt == 0:
                # kt-outer / mt-inner: all 8 PSUM banks accumulate in
                # parallel, so the PE starts as soon as the first A/W k'-tiles
                # are cast and stays busy while the A-cast ramp completes.
                ps_tiles = [
                    pspool.tile([128, 512], mybir.dt.float32, tag="ps", name="ps")
                    for _ in range(mt_n)
                ]
                for kt in range(kt_n):
                    for mt in range(mt_n):
                        nc.tensor.matmul(
                            ps_tiles[mt][:, :nw],
                            a_all[:, kt * m + mt * 128 : kt * m + mt * 128 + 128],
                            w_all[:, kt * 512 : kt * 512 + nw],
                            start=(kt == 0),
                            stop=(kt == kt_n - 1),
                        )
                for mt in range(mt_n):
                    o = opool.tile([128, 512], mybir.dt.int32, tag="o")
                    if mt % 2 == 0:
                        nc.vector.tensor_copy(o[:, :nw], ps_tiles[mt][:, :nw])
                    else:
                        nc.scalar.copy(o[:, :nw], ps_tiles[mt][:, :nw])
                    nc.sync.dma_start(
                        c[mt * 128 : (mt + 1) * 128, n0 : n0 + nw], o[:, :nw]
                    )
            else:
                # Steady state (A resident, W prefetched): mt-outer so each
                # m-tile's PSUM copy + store streams out while the next
                # m-tile's matmuls run, instead of bursting at the tile end.
                for mt in range(mt_n):
                    ps = pspool.tile([128, 512], mybir.dt.float32, tag="ps", name="ps")
                    for kt in range(kt_n):
                        nc.tensor.matmul(
                            ps[:, :nw],
                            a_all[:, kt * m + mt * 128 : kt * m + mt * 128 + 128],
                            w_all[:, kt * 512 : kt * 512 + nw],
                            start=(kt == 0),
                            stop=(kt == kt_n - 1),
                        )
                    o = opool.tile([128, 512], mybir.dt.int32, tag="o")
                    if mt % 2 == 0:
                        nc.vector.tensor_copy(o[:, :nw], ps[:, :nw])
                    else:
                        nc.scalar.copy(o[:, :nw], ps[:, :nw])
                    nc.sync.dma_start(
                        c[mt * 128 : (mt + 1) * 128, n0 : n0 + nw], o[:, :nw]
                    )

    nc.compile()
    return nc


def _get_program():
    key = (M, K, NSHARD, NCORES)
    if key not in _prog_cache:
        _prog_cache[key] = _build(*key)
    return _prog_cache[key]


def _prep_inputs(A, B):
    A = np.ascontiguousarray(np.asarray(A, dtype=np.int8))
    B = np.ascontiguousarray(np.asarray(B, dtype=np.int8))
    # A^T with k-permutation sigma(k' = i*(K/4) + 4g + j) = 16g + 4i + j.
    a_perm_t = np.ascontiguousarray(
        A.reshape(M, K // 16, 4, 4).transpose(2, 1, 3, 0).reshape(K, M)
    )
    b_t = np.ascontiguousarray(B.T)  # [K//4, N] int8
    return a_perm_t, b_t


def kernel(A, B):
    from concourse.bass_utils import run_bass_kernel_spmd

    a_perm_t, b_t = _prep_inputs(A, B)
    nc = _get_program()
    in_maps = [
        {
            "a_t": a_perm_t,
            "b_t": np.ascontiguousarray(
                b_t[:, ci * NSHARD : (ci + 1) * NSHARD]
            ).view(np.int32),
        }
        for ci in range(NCORES)
    ]
    res = run_bass_kernel_spmd(nc, in_maps, core_ids=list(range(NCORES)))
    return np.concatenate([res.results[ci]["c"] for ci in range(NCORES)], axis=1)
